# revision 7
# baseline (speedup 1.0000x reference)
"""Trainium2 Bass kernel for CasAttention2D.

Math (reference):
    kh  = k @ Wk;  v = kh @ Wv;  qh = q @ Wq
    ph  = relu(pos @ P1 + pb1) @ P2 + pb2
    s   = kh - qh[:,:,None,:] + ph
    a   = relu(s @ A1 + ab1) @ A2 + ab2
    a   = where(mask==0, -1e9, a); attn = softmax(a, axis=K)
    out = ((v + ph) * attn).sum(K) @ Wo + bo

Device-side reformulation (per token-row r = (token, k)):
    kmq = k - q_broadcast                       (host fold)
    vph = kmq @ (Wk Wv) + relu(pos@P1+pb1) @ P2 + q @ (Wk Wv) + pb2
    s1  = kmq @ (Wk A1) + relu(pos@P1+pb1) @ (P2 A1) + (ab1 + pb2@A1)
    a2  = relu(s1) @ A2 + madd                  (ab2 dropped: softmax-invariant)
    num = exp(a2);  den = segsum_K(num)
    out_f = Wo^T (segsum_K(vph*num) / den) + bo

Everything on-device runs feature-major: SBUF tiles are [feature, row].
The host transposes inputs once and transposes the output back.
"""

import numpy as np
from contextlib import ExitStack

import sys

for _p in ("/root/.axon_site/_ro/trn_rl_repo", "/root/.axon_site/_ro/pypackages",
           "/opt/trn_rl_repo", "/opt/pypackages"):
    if _p not in sys.path:
        sys.path.append(_p)

import concourse.bass as bass
import concourse.tile as tile
from concourse import mybir
from concourse.bass_utils import run_bass_kernel_spmd

# problem dims (hardcoded per contract)
B, N, K, D = 4, 4096, 16, 128
H = D // 8
NCORES = 8
T_TOTAL = B * N                 # 16384 tokens
T_CORE = T_TOTAL // NCORES      # 2048 tokens per core
R_CORE = T_CORE * K             # 32768 k-rows per core
CHUNK = 512                     # k-rows per chunk (32 tokens)
TOK_CHUNK = CHUNK // K          # 32 tokens per chunk
NCHUNK = R_CORE // CHUNK        # 64
GRP = 8                         # chunks per output group (256 tokens)
TOK_GRP = GRP * TOK_CHUNK       # 256

F32 = mybir.dt.float32
F32R = mybir.dt.float32r
AF = mybir.ActivationFunctionType
ALU = mybir.AluOpType


def _legalize_waits(nc):
    """This walrus build encodes at most ONE sync-wait per instruction.
    Split multi-wait instructions into single-wait same-engine NoOps."""
    cnt = 0
    for fn in nc.m.functions:
        for blk in fn.blocks:
            bb = blk.bb if hasattr(blk, "bb") else blk
            insts = bb.instructions
            new_list = []
            for inst in insts:
                si = inst.sync_info
                waits = list(si.on_wait) if (si and si.on_wait) else []
                if len(waits) > 1:
                    for w in waits[:-1]:
                        cnt += 1
                        nop = mybir.InstNoOp(
                            name=f"WSPLIT-{cnt}-{inst.name}",
                            sync_info=mybir.SyncInfo(on_wait=[w], on_update=[]),
                        )
                        nop.engine = inst.engine
                        new_list.append(nop)
                    si.on_wait = [waits[-1]]
                new_list.append(inst)
            del insts[:]
            for x in new_list:
                insts.append(x)
    return cnt


def _build_program(uadd_chunks):
    """Build the SPMD Bass program. uadd_chunks: set of chunk indices that
    need the all-masked-token uniform-leak correction."""
    nc = bass.Bass()

    # per-core DRAM inputs (feature-major)
    kf = nc.dram_tensor("kf", [D, R_CORE], F32R, kind="ExternalInput")
    posf = nc.dram_tensor("posf", [4, R_CORE], F32R, kind="ExternalInput")
    qf = nc.dram_tensor("qf", [D, T_CORE], F32R, kind="ExternalInput")
    madd = nc.dram_tensor("madd", [1, R_CORE], F32R, kind="ExternalInput")
    uadd = nc.dram_tensor("uadd", [1, R_CORE], F32, kind="ExternalInput")

    w_kv = nc.dram_tensor("w_kv", [D, D], F32R, kind="ExternalInput")
    w_ka = nc.dram_tensor("w_ka", [D, H], F32R, kind="ExternalInput")
    w_p1 = nc.dram_tensor("w_p1", [4, H], F32R, kind="ExternalInput")
    w_p2 = nc.dram_tensor("w_p2", [H, D], F32R, kind="ExternalInput")
    w_p2a = nc.dram_tensor("w_p2a", [H, H], F32R, kind="ExternalInput")
    w_a2 = nc.dram_tensor("w_a2", [H, D], F32R, kind="ExternalInput")
    w_o = nc.dram_tensor("w_o", [D, D], F32, kind="ExternalInput")
    w_ones = nc.dram_tensor("w_ones", [1, D], F32R, kind="ExternalInput")
    w_nqa = nc.dram_tensor("w_nqa", [D, H], F32R, kind="ExternalInput")
    b_p1 = nc.dram_tensor("b_p1", [H, 1], F32, kind="ExternalInput")
    b_s1 = nc.dram_tensor("b_s1", [H, 1], F32, kind="ExternalInput")
    b_p2 = nc.dram_tensor("b_p2", [D, 1], F32, kind="ExternalInput")
    b_o = nc.dram_tensor("b_o", [D, 1], F32, kind="ExternalInput")

    out_f = nc.dram_tensor("out_f", [D, T_CORE], F32, kind="ExternalOutput")

    with ExitStack() as ctx:
        tc = ctx.enter_context(tile.TileContext(nc))
        consts = ctx.enter_context(tc.tile_pool(name="consts", bufs=1))
        kpool = ctx.enter_context(tc.tile_pool(name="kpool", bufs=3))
        spool = ctx.enter_context(tc.tile_pool(name="spool", bufs=3))
        vpool = ctx.enter_context(tc.tile_pool(name="vpool", bufs=3))
        dpool = ctx.enter_context(tc.tile_pool(name="dpool", bufs=4))
        gpool = ctx.enter_context(tc.tile_pool(name="gpool", bufs=2))
        ps_misc = ctx.enter_context(tc.tile_pool(name="ps_misc", bufs=1, space="PSUM"))
        ps_p1 = ctx.enter_context(tc.tile_pool(name="ps_p1", bufs=2, space="PSUM"))
        ps_s1 = ctx.enter_context(tc.tile_pool(name="ps_s1", bufs=2, space="PSUM"))
        ps_vph = ctx.enter_context(tc.tile_pool(name="ps_vph", bufs=2, space="PSUM"))
        ps_a2 = ctx.enter_context(tc.tile_pool(name="ps_a2", bufs=1, space="PSUM"))

        # load weights/biases once (distinct tags: one resident slot each)
        def wtile(dram, shape, dt=F32R):
            t = consts.tile(shape, dt, tag=f"w_{dram.name}")
            nc.sync.dma_start(out=t, in_=dram[:])
            return t

        Wkv = wtile(w_kv, [D, D])
        Wka = wtile(w_ka, [D, H])
        P1 = wtile(w_p1, [4, H])
        P2 = wtile(w_p2, [H, D])
        P2a = wtile(w_p2a, [H, H])
        A2 = wtile(w_a2, [H, D])
        Wo = wtile(w_o, [D, D], F32)
        Ones1 = wtile(w_ones, [1, D])
        NQa = wtile(w_nqa, [D, H])
        Bp1 = wtile(b_p1, [H, 1], F32)
        Bs1 = wtile(b_s1, [H, 1], F32)
        Bp2 = wtile(b_p2, [D, 1], F32)
        Bo = wtile(b_o, [D, 1], F32)

        for c in range(NCHUNK):
            g = c // GRP
            ci = c % GRP
            r0 = c * CHUNK
            t0 = ci * TOK_CHUNK  # token offset within group

            if ci == 0:
                # per-group q tile + qv = Wkv^T q (+ pb2) in SBUF
                qt = gpool.tile([D, TOK_GRP], F32R, tag="qt")
                nc.sync.dma_start(out=qt, in_=qf[:, g * TOK_GRP:(g + 1) * TOK_GRP])
                # per-group output accumulator
                xsup = gpool.tile([D, TOK_GRP], F32, tag="xsup")

            kt = kpool.tile([D, CHUNK], F32R, tag="kmq")
            nc.sync.dma_start(out=kt, in_=kf[:, r0:r0 + CHUNK])
            post = kpool.tile([4, CHUNK], F32R, tag="pos")
            nc.sync.dma_start(out=post, in_=posf[:, r0:r0 + CHUNK])
            maddt = kpool.tile([1, CHUNK], F32R, tag="madd")
            nc.sync.dma_start(out=maddt, in_=madd[:, r0:r0 + CHUNK])

            # pos MLP first layer
            p1_ps = ps_p1.tile([H, CHUNK], F32, tag="p1")
            nc.tensor.matmul(p1_ps[:], P1[:], post[:], start=True, stop=True)
            r1 = spool.tile([H, CHUNK], F32R, tag="r1")
            nc.vector.tensor_scalar(out=r1[:], in0=p1_ps[:], scalar1=Bp1[:],
                                    scalar2=0.0, op0=ALU.add, op1=ALU.max)

            # attention-MLP hidden pre-act
            s1_ps = ps_s1.tile([H, CHUNK], F32, tag="s1")
            nc.tensor.matmul(s1_ps[:], Wka[:], kt[:], start=True, stop=False)
            nc.tensor.matmul(s1_ps[:], P2a[:], r1[:], start=False, stop=False)
            qb = qt[:, t0:t0 + TOK_CHUNK].unsqueeze(2).broadcast_to(
                (D, TOK_CHUNK, K))
            nc.tensor.matmul(s1_ps[:], NQa[:], qb, start=False, stop=True)
            a1 = spool.tile([H, CHUNK], F32R, tag="a1")
            nc.vector.tensor_scalar(out=a1[:], in0=s1_ps[:], scalar1=Bs1[:],
                                    scalar2=0.0, op0=ALU.add, op1=ALU.max)

            # values v+ph (PSUM-accumulated)
            vph_ps = ps_vph.tile([D, CHUNK], F32, tag="vph")
            nc.tensor.matmul(vph_ps[:], Wkv[:], kt[:], start=True, stop=False)
            nc.tensor.matmul(vph_ps[:], P2[:], r1[:], start=False, stop=True)

            # logits + additive mask (PSUM-accumulated)
            a2_ps = ps_a2.tile([D, CHUNK], F32, tag="a2")
            nc.tensor.matmul(a2_ps[:], A2[:], a1[:], start=True, stop=False)
            nc.tensor.matmul(a2_ps[:], Ones1[:], maddt[:], start=False, stop=True)

            num = vpool.tile([D, CHUNK], F32, tag="num")
            nc.scalar.activation(num[:], a2_ps[:], AF.Exp)
            if c in uadd_chunks:
                uaddt = kpool.tile([1, CHUNK], F32, tag="uadd")
                nc.sync.dma_start(out=uaddt, in_=uadd[:, r0:r0 + CHUNK])
                ub = uaddt[:].partition_broadcast(D).rearrange("p q f -> p (q f)")
                nc.vector.tensor_tensor(out=num[:], in0=num[:], in1=ub,
                                        op=ALU.add)

            den = dpool.tile([D, TOK_CHUNK], F32, tag="den")
            nc.vector.tensor_reduce(out=den[:], in_=num[:].rearrange(
                "p (a b) -> p a b", b=K), axis=mybir.AxisListType.X, op=ALU.add)
            rec = dpool.tile([D, TOK_CHUNK], F32, tag="rec")
            nc.vector.reciprocal(out=rec[:], in_=den[:])

            # vph to SBUF with per-feature pb2 bias
            vph = vpool.tile([D, CHUNK], F32, tag="vphsb")
            nc.vector.tensor_scalar(out=vph[:], in0=vph_ps[:], scalar1=Bp2[:],
                                    scalar2=None, op0=ALU.add)

            y = vpool.tile([D, CHUNK], F32, tag="y")
            nc.vector.tensor_tensor(out=y[:], in0=vph[:], in1=num[:],
                                    op=ALU.mult)
            ynum = dpool.tile([D, TOK_CHUNK], F32, tag="ynum")
            nc.vector.tensor_reduce(out=ynum[:], in_=y[:].rearrange(
                "p (a b) -> p a b", b=K), axis=mybir.AxisListType.X, op=ALU.add)
            nc.vector.tensor_tensor(out=xsup[:, t0:t0 + TOK_CHUNK],
                                    in0=ynum[:], in1=rec[:], op=ALU.mult)

            if ci == GRP - 1:
                wo_ps = ps_misc.tile([D, TOK_GRP], F32, tag="misc")
                nc.tensor.matmul(wo_ps[:], Wo[:], xsup[:],
                                 start=True, stop=True)
                outt = gpool.tile([D, TOK_GRP], F32, tag="outt")
                nc.scalar.activation(outt[:], wo_ps[:], AF.Identity, bias=Bo[:])
                nc.sync.dma_start(out=out_f[:, g * TOK_GRP:(g + 1) * TOK_GRP],
                                  in_=outt[:])

    _legalize_waits(nc)
    return nc


_CACHE = {}


def kernel(q, k, pos, mask, Wq, Wk, Wv, P1, pb1, P2, pb2,
           A1, ab1, A2, ab2, Wo, bo):
    q = np.asarray(q, np.float32)
    k = np.asarray(k, np.float32)
    pos = np.asarray(pos, np.float32)
    mask_np = np.asarray(mask)
    Wq, Wk, Wv = (np.asarray(x, np.float32) for x in (Wq, Wk, Wv))
    P1, pb1, P2, pb2 = (np.asarray(x, np.float32) for x in (P1, pb1, P2, pb2))
    A1, ab1, A2, ab2 = (np.asarray(x, np.float32) for x in (A1, ab1, A2, ab2))
    Wo, bo = np.asarray(Wo, np.float32), np.asarray(bo, np.float32)

    # ---- host-side input prep (layout + weight folding) ----
    kT = np.ascontiguousarray(k.reshape(T_TOTAL * K, D).T)     # [D, R]
    posT = np.ascontiguousarray(pos.reshape(T_TOTAL * K, 4).T)  # [4, R]
    qT = np.ascontiguousarray(q.reshape(T_TOTAL, D).T)  # [D, T]
    m = mask_np.reshape(T_TOTAL, K) != 0
    maddv = np.where(m, np.float32(0), np.float32(-1e9)).reshape(1, -1)
    all_masked = ~m.any(axis=1)                         # [T]
    uaddv = np.repeat(all_masked.astype(np.float32), K).reshape(1, -1)

    w_kv = np.ascontiguousarray(Wk @ Wv)
    w_ka = np.ascontiguousarray(Wk @ A1)
    w_p2a = np.ascontiguousarray(P2 @ A1)
    w_nqa = np.ascontiguousarray(-(Wq @ A1))
    b_s1 = (ab1 + pb2 @ A1).reshape(H, 1)
    w_ones = np.ones((1, D), np.float32)

    # which chunks need the uniform-leak correction (per core -> global union;
    # SPMD shares one program, so apply the union of chunk indices)
    uadd_chunks = set()
    if all_masked.any():
        idx = np.nonzero(all_masked)[0]
        for t in idx:
            core = t // T_CORE
            local_tok = t - core * T_CORE
            uadd_chunks.add(local_tok // TOK_CHUNK)

    key = ("v2", tuple(sorted(uadd_chunks)))
    if key not in _CACHE:
        _CACHE[key] = _build_program(uadd_chunks)
    nc = _CACHE[key]

    shared = {
        "w_kv": w_kv, "w_ka": w_ka, "w_p1": P1, "w_p2": np.ascontiguousarray(P2),
        "w_p2a": w_p2a, "w_a2": np.ascontiguousarray(A2),
        "w_o": np.ascontiguousarray(Wo), "w_ones": w_ones, "w_nqa": w_nqa,
        "b_p1": pb1.reshape(H, 1), "b_s1": b_s1,
        "b_p2": pb2.reshape(D, 1), "b_o": bo.reshape(D, 1),
    }
    in_maps = []
    for c in range(NCORES):
        rs, re = c * R_CORE, (c + 1) * R_CORE
        ts, te = c * T_CORE, (c + 1) * T_CORE
        im = dict(shared)
        im["kf"] = np.ascontiguousarray(kT[:, rs:re])
        im["posf"] = np.ascontiguousarray(posT[:, rs:re])
        im["qf"] = np.ascontiguousarray(qT[:, ts:te])
        im["madd"] = np.ascontiguousarray(maddv[:, rs:re])
        im["uadd"] = np.ascontiguousarray(uaddv[:, rs:re])
        in_maps.append(im)

    res = run_bass_kernel_spmd(nc, in_maps, core_ids=list(range(NCORES)))
    kernel._last_results = res
    out = np.concatenate([res.results[c]["out_f"] for c in range(NCORES)],
                         axis=1)                        # [D, T]
    return np.ascontiguousarray(out.T).reshape(B, N, D).astype(np.float32)
